# revision 9
# baseline (speedup 1.0000x reference)
"""Trainium2 Bass kernel for a Bayesian (variational) single-layer LSTM.

Reference computation (B=512, S=128, H=512, IN=1, OUT=1):
    W = mu + softplus(rho) * eps            (variational reparameterization)
    u[b,s] = x[b,s] * mask_in[b,s]          (inverted input dropout)
    gates(t) = u[:,t] * W_ih + b + h(t-1) @ W_hh
    i,f,g,o = split(gates); c = f*c + i*g; h = o * tanh(c)
    out = (h(S-1) * mask_out) @ W_lin + b_lin

Strategy: data-parallel over batch (64 rows per core, 8 cores), weights
replicated.  Feature-major layout on chip: gates^T / h^T / c^T with gate
features on partitions, batch on the free dim.

    gates^T[128-feat tile m, batch] = sum_k W_hh[k-chunk, m-tile].T @ h^T[k-chunk]
                                      (+ [W_ih; b].T @ [u_s; 1])

PSUM layout (the key restructure vs the per-gate baseline): per feature
HALF j (h chunks 2j,2j+1) one full PSUM bank holds [i_j|f_j|g_j|o_j]
(each 128 cols = 2 chunks x 64 batch), so ONE sigmoid instruction per
half covers all four gates.  tanh(x)=2*sigmoid(2x)-1 scaling is folded
into the weights (g columns x2) and h is stored as h/2 (all W columns
x2, W_lin x2), so a single SIGMOID table serves the gates; the c->h
tail uses TANH (same ACT table set) with scale=2:  h/2 = 0.5*tanh(2C)*o.

PE ordering is just-in-time: per step, gx (input projection) and the
k-chunks of the EARLY h half issue first, the late half's k-chunks
last, so sigma of half 0 starts after only 16 trailing matmuls and the
sigma/c/tanh/h chain of each half overlaps the other half's matmuls.

Precision: W/h/u fp16, PSUM fp32, sigma outputs fp32, c fp32, T fp16.
"""

import os
import sys

import numpy as np

for _p in ("/opt/trn_rl_repo",):
    if _p not in sys.path:
        sys.path.insert(0, _p)

from concourse import bacc, bass, mybir, tile  # noqa: E402
from concourse.bass_utils import run_bass_kernel_spmd  # noqa: E402
from concourse.tile_rust import add_dep_helper  # noqa: E402

N_CORES = 8
B, S, H, OUT = 512, 128, 512, 1
BL = B // N_CORES            # 64 batch rows per core
G4 = 4 * H                   # 2048 gate features
KC = H // 128                # 4 contraction chunks
F32 = mybir.dt.float32
F16 = mybir.dt.float16
AF = mybir.ActivationFunctionType
OP = mybir.AluOpType

_cache = {}


def _build():
    if "built" in _cache:
        return _cache["built"]

    nc = bacc.Bacc(
        "TRN2", target_bir_lowering=False, debug=False, num_devices=N_CORES
    )

    # ---- I/O ----
    def din(name, shape):
        return nc.dram_tensor(name, shape, F32, kind="ExternalInput").ap()

    x_sl = din("x_sl", [BL, S])
    mk_sl = din("mk_sl", [BL, S])
    mo_sl = din("mo_sl", [BL, H])
    wih_mu, wih_rho, eps_ih = din("wih_mu", [1, G4]), din("wih_rho", [1, G4]), din("eps_ih", [1, G4])
    b_mu, b_rho, eps_b = din("b_mu", [1, G4]), din("b_rho", [1, G4]), din("eps_b", [1, G4])
    whh_mu, whh_rho, eps_hh = din("whh_mu", [H, G4]), din("whh_rho", [H, G4]), din("eps_hh", [H, G4])
    wlin = din("wlin", [H, OUT])
    blin = din("blin", [1, OUT])
    out_d = nc.dram_tensor("out", [BL, OUT], F32, kind="ExternalOutput").ap()
    u_scr = nc.dram_tensor("u_scr", [S, BL], F16, kind="Internal").ap()

    with tile.TileContext(nc) as tc:
        with tc.tile_pool(name="const", bufs=1) as const:
            w16 = [
                const.tile([128, G4], F16, tag=f"w16_{k}", name=f"w16_{k}")
                for k in range(KC)
            ]
            wg = const.tile([2, G4], F16, tag="wg", name="wg")
            u2 = const.tile([2, S * BL], F16, tag="u2", name="u2")
            mot = const.tile([128, KC * BL], F16, tag="mot", name="mot")
            wl16 = const.tile([128, KC], F16, tag="wl16", name="wl16")
            bl32 = const.tile([1, OUT], F32, tag="bl32", name="bl32")

            # ------------- prologue (input path first, then weights) -------
            with tc.tile_pool(name="pre", bufs=2) as pre:
                # u = x * mask_in -> transposed + flattened U2 [2, S*BL]
                xt = pre.tile([BL, S], F32, tag="xt", name="xt")
                mkt = pre.tile([BL, S], F32, tag="mkt", name="mkt")
                nc.sync.dma_start(xt[:, :], x_sl)
                nc.sync.dma_start(mkt[:, :], mk_sl)
                u16 = pre.tile([BL, S], F16, tag="u16", name="u16")
                nc.vector.tensor_mul(u16[:, :], xt[:, :], mkt[:, :])
                ut = pre.tile([S, BL], F16, tag="ut", name="ut")
                nc.sync.dma_start_transpose(ut[:, :], u16[:, :])
                nc.sync.dma_start(u_scr, ut[:, :])
                nc.sync.dma_start(
                    u2[0:1, :], u_scr.rearrange("s b -> (s b)")[None, :]
                )
                ones_row = pre.tile([1, S * BL], F16, tag="ones_row", name="ones_row")
                nc.gpsimd.memset(ones_row[:, :], 1.0)
                nc.sync.dma_start(u2[1:2, :], ones_row[:, :])

                # scale patterns: tanh(x)=2*sigmoid(2x)-1 trick needs the
                # g-gate pre-activations doubled; storing h/2 needs all
                # W_hh columns doubled (and W_lin doubled at the output).
                sc_row = pre.tile([1, G4], F32, tag="sc_row", name="sc_row", bufs=1)
                nc.gpsimd.memset(sc_row[:, :], 1.0)
                nc.gpsimd.memset(sc_row[:, 1024:1536], 2.0)
                sc_w = pre.tile([128, G4], F32, tag="sc_w", name="sc_w", bufs=1)
                nc.gpsimd.memset(sc_w[:, :], 2.0)
                nc.gpsimd.memset(sc_w[:, 1024:1536], 4.0)

                # ---- sampling, phase-batched so each ACT table set loads
                # once: all Exp ops, then all Ln ops (sets differ), then the
                # DVE chain per unit.  rho is DMA'd first so Exp starts early.
                units = [("wih", wih_rho, wih_mu, eps_ih, 1, sc_row),
                         ("bb", b_rho, b_mu, eps_b, 1, sc_row)]
                for k in range(KC):
                    rsl = slice(128 * k, 128 * (k + 1))
                    units.append((f"whh{k}", whh_rho[rsl, :], whh_mu[rsl, :],
                                  eps_hh[rsl, :], 128, sc_w))
                exs = []
                for nm, rho_ap, mu_ap, eps_ap, rows, sc in units:
                    rho = pre.tile([rows, G4], F32, tag="smp_rho",
                                   name=f"{nm}_rho")
                    nc.sync.dma_start(rho[:, :], rho_ap)
                    ex = pre.tile([rows, G4], F32, tag=f"ex_{nm}",
                                  name=f"{nm}_ex", bufs=1)
                    nc.scalar.activation(ex[:, :], rho[:, :], AF.Exp)
                    exs.append(ex)
                for ex, u in zip(exs, units):
                    nc.scalar.activation(ex[:, :], ex[:, :], AF.Ln, bias=1.0)
                outs = []
                for ex, (nm, rho_ap, mu_ap, eps_ap, rows, sc) in zip(exs, units):
                    eps = pre.tile([rows, G4], F32, tag="smp_eps",
                                   name=f"{nm}_eps")
                    mu = pre.tile([rows, G4], F32, tag="smp_mu", name=f"{nm}_mu")
                    nc.sync.dma_start(eps[:, :], eps_ap)
                    nc.sync.dma_start(mu[:, :], mu_ap)
                    nc.vector.tensor_mul(ex[:, :], ex[:, :], eps[:, :])
                    nc.vector.tensor_add(ex[:, :], ex[:, :], mu[:, :])
                    if rows == 1:
                        o = pre.tile([1, G4], F16, tag="wrow", name=f"{nm}_16")
                        nc.vector.tensor_mul(o[:, :], ex[:, :], sc[:, :])
                        outs.append(o)
                    else:
                        kk = int(nm[3:])
                        nc.vector.tensor_mul(w16[kk][:, :], ex[:, :], sc[:, :])
                nc.sync.dma_start(wg[0:1, :], outs[0][:, :])
                nc.sync.dma_start(wg[1:2, :], outs[1][:, :])

                # mask_out^T fp16, W_lin fp16, b_lin
                mo32 = pre.tile([BL, H], F32, tag="mo32", name="mo32")
                nc.sync.dma_start(mo32[:, :], mo_sl)
                mo16 = pre.tile([BL, H], F16, tag="mo16", name="mo16")
                nc.gpsimd.tensor_copy(mo16[:, :], mo32[:, :])
                for k in range(KC):
                    nc.sync.dma_start_transpose(
                        mot[:, BL * k:BL * (k + 1)], mo16[:, 128 * k:128 * (k + 1)]
                    )
                wl32 = pre.tile([128, KC], F32, tag="wl32", name="wl32")
                for k in range(KC):
                    nc.sync.dma_start(
                        wl32[:, k:k + 1], wlin[128 * k:128 * (k + 1), :]
                    )
                nc.gpsimd.tensor_scalar_mul(wl16[:, :], wl32[:, :], 2.0)
                nc.sync.dma_start(bl32[:, :], blin)

            # ------------- recurrence -------------
            # PSUM: one full bank per feature half j: [i_j|f_j|g_j|o_j],
            # each gate block 128 cols = chunks (2j,2j+1) x 64 batch.
            # m-tile m (gate gt=m//4, chunk q=m%4) -> bank q//2,
            # cols gt*128 + (q%2)*64.
            n_fill = int(os.environ.get("KERNEL_FILL", "14"))
            with tc.tile_pool(name="work", bufs=4) as work:
              with tc.tile_pool(name="psum", bufs=2, space="PSUM") as psum:
                fill_ps = psum.tile([128, BL], F32, tag="pfill", name="pfill",
                                    bufs=1)
                h_prev = None
                c_prev = [None, None]
                for s in range(S):
                    bank = [
                        psum.tile([128, 512], F32, tag=f"pb{j}", name=f"pb{j}_{s}")
                        for j in (0, 1)
                    ]
                    u_s = u2[:, BL * s:BL * (s + 1)]

                    def out_ap(m):
                        gt, q = m // 4, m % 4
                        col = gt * 128 + (q % 2) * 64
                        return bank[q // 2][:, col:col + 64]

                    def gx_mm(m, start):
                        return nc.tensor.matmul(
                            out_ap(m),
                            wg[:, 128 * m:128 * (m + 1)],
                            u_s,
                            start=start, stop=False, skip_group_check=True,
                        )

                    def k_mm(k, m):
                        hsrc = h_prev[k // 2][:, 64 * (k % 2):64 * (k % 2 + 1)]
                        return nc.tensor.matmul(
                            out_ap(m),
                            w16[k][:, 128 * m:128 * (m + 1)],
                            hsrc,
                            start=False, stop=(k == KC - 1),
                            skip_group_check=True,
                        )

                    # gx: one opener per bank, rest accumulate.
                    openers = {}
                    for j in (0, 1):
                        m0 = 2 * j            # (gt=0, q=2j) lives in bank j
                        openers[j] = gx_mm(m0, True)
                    for m in range(16):
                        if m in (0, 2):
                            continue
                        r = gx_mm(m, False)
                        add_dep_helper(
                            r.ins, openers[(m % 4) // 2].ins,
                            reason="bank start first",
                        )
                    if h_prev is not None:
                        # JIT order: kA (chunks 0,1; early h half) for the
                        # late-half-1 m-tiles first, then half-0; kB (chunks
                        # 2,3; late h half) for half-1 first so sigma_1 --
                        # the critical chain -- unblocks after only 16
                        # trailing matmuls once h1(s-1) lands.
                        for j in (1, 0):
                            for k in (0, 1):
                                for m in range(16):
                                    if (m % 4) // 2 != j:
                                        continue
                                    r = k_mm(k, m)
                                    add_dep_helper(
                                        r.ins, openers[j].ins,
                                        reason="bank start first",
                                    )
                        for j in (1, 0):
                            for k in (2, 3):
                                for m in range(16):
                                    if (m % 4) // 2 != j:
                                        continue
                                    r = k_mm(k, m)
                                    add_dep_helper(
                                        r.ins, openers[j].ins,
                                        reason="bank start first",
                                    )
                    # step 0: gates are gx only; groups stay open (harmless
                    # with skip_group_check, matching the baseline pattern)

                    # p-state filler: the PE DVFS drops to half speed after
                    # idle gaps (>3us continuous busy needed for full clock).
                    # Cheap throwaway matmuls into a scratch bank keep the
                    # engine busy through the sigma/c/tanh chain latency so
                    # real matmuls run at ~29ns instead of ~53ns.
                    for _f in range(n_fill):
                        nc.tensor.matmul(
                            fill_ps[:, :],
                            wg[:, 0:128],
                            u2[:, 0:BL],
                            start=True, stop=True, skip_group_check=True,
                        )

                    # one sigmoid per half over [i|f|g|o] (fp32 out);
                    # half 1 (the late h half / critical chain) first
                    sg = [None, None]
                    for j in (1, 0):
                        sg[j] = work.tile(
                            [128, 512], F32, tag=f"sg{j}", name=f"sg{j}_{s}",
                            bufs=2,
                        )
                        nc.scalar.activation(
                            sg[j][:, :], bank[j][:, :], AF.Sigmoid
                        )

                    # DVE chain per half: fc = sf*c', t = (sg-.5)*si,
                    # c = t+fc; tail: T = tanh(2C) fp16, H = (T*.5)*so fp16
                    h_new = [
                        work.tile([128, 128], F16, tag=f"hT{j}",
                                  name=f"hT{j}_{s}", bufs=2)
                        for j in (0, 1)
                    ]
                    c_new = [None, None]
                    t_t = [None, None]
                    fc_t = [None, None]
                    T_t = [None, None]

                    def chain_mul(j):
                        si = sg[j][:, 0:128]
                        sf = sg[j][:, 128:256]
                        sgg = sg[j][:, 256:384]
                        t_t[j] = work.tile(
                            [128, 128], F32, tag=f"t{j}", name=f"t{j}_{s}",
                            bufs=2,
                        )
                        if c_prev[j] is not None:
                            fc_t[j] = work.tile(
                                [128, 128], F32, tag=f"fc{j}", name=f"fc{j}_{s}",
                                bufs=2,
                            )
                            nc.vector.tensor_mul(
                                fc_t[j][:, :], sf, c_prev[j][:, :]
                            )
                        nc.vector.scalar_tensor_tensor(
                            t_t[j][:, :], sgg, 0.5, si,
                            op0=OP.subtract, op1=OP.mult,
                        )

                    def chain_add(j):
                        if c_prev[j] is None:
                            c_new[j] = t_t[j]
                            return
                        c_new[j] = work.tile(
                            [128, 128], F32, tag=f"cT{j}", name=f"cT{j}_{s}",
                            bufs=2,
                        )
                        nc.vector.tensor_add(
                            c_new[j][:, :], t_t[j][:, :], fc_t[j][:, :]
                        )

                    def chain_tanh(j):
                        T_t[j] = work.tile(
                            [128, 128], F16, tag=f"T{j}", name=f"T{j}_{s}",
                            bufs=2,
                        )
                        nc.scalar.activation(
                            T_t[j][:, :], c_new[j][:, :], AF.Tanh, scale=2.0
                        )

                    def chain_h(j):
                        so = sg[j][:, 384:512]
                        nc.vector.scalar_tensor_tensor(
                            h_new[j][:, :], T_t[j][:, :],
                            0.5, so, op0=OP.mult, op1=OP.mult,
                        )

                    # program order tuned for overlap (half 1 = critical):
                    # DVE: fc1,t1,c1, fc0,t0, h1, c0, h0
                    # ACT: sig1, sig0, T1, T0
                    chain_mul(1)
                    chain_add(1)
                    chain_tanh(1)
                    chain_mul(0)
                    chain_h(1)
                    chain_add(0)
                    chain_tanh(0)
                    chain_h(0)
                    c_prev = c_new
                    h_prev = h_new

              # ------------- epilogue (psum pool released; reuse banks) ----
              with tc.tile_pool(name="psum2", bufs=1, space="PSUM") as psum2:
                mh = work.tile([128, KC * BL], F16, tag="mh", name="mh")
                for j in (0, 1):
                    nc.vector.tensor_mul(
                        mh[:, 128 * j:128 * (j + 1)], h_prev[j][:, :],
                        mot[:, 128 * j:128 * (j + 1)],
                    )
                pso = psum2.tile([1, BL], F32, tag="pso", name="pso", bufs=1)
                for k in range(KC):
                    nc.tensor.matmul(
                        pso[0:1, :],
                        wl16[:, k:k + 1],
                        mh[:, BL * k:BL * (k + 1)],
                        start=(k == 0), stop=(k == KC - 1),
                    )
                osb = work.tile([1, BL], F32, tag="osb", name="osb")
                nc.vector.tensor_scalar(
                    osb[:, :], pso[0:1, :], bl32[0:1, 0:1], None, op0=OP.add
                )
                nc.sync.dma_start(out_d.rearrange("b o -> o b"), osb[:, :])

    nc.compile()
    _cache["built"] = nc
    return nc


def kernel(**inputs) -> np.ndarray:
    nc = _build()
    f32 = np.float32

    def c(a):
        return np.ascontiguousarray(np.asarray(a, dtype=f32))

    shared = {
        "wih_mu": c(inputs["W_ih_mu"]).reshape(1, G4),
        "wih_rho": c(inputs["W_ih_rho"]).reshape(1, G4),
        "eps_ih": c(inputs["eps_ih"]).reshape(1, G4),
        "b_mu": c(inputs["b_mu"]).reshape(1, G4),
        "b_rho": c(inputs["b_rho"]).reshape(1, G4),
        "eps_b": c(inputs["eps_b"]).reshape(1, G4),
        "whh_mu": c(inputs["W_hh_mu"]),
        "whh_rho": c(inputs["W_hh_rho"]),
        "eps_hh": c(inputs["eps_hh"]),
        "wlin": c(inputs["W_lin"]).reshape(H, OUT),
        "blin": c(inputs["b_lin"]).reshape(1, OUT),
    }
    x = c(inputs["x"])
    mk = c(inputs["mask_in"]).reshape(B, S)
    mo = c(inputs["mask_out"])
    in_maps = []
    for i in range(N_CORES):
        sl = slice(BL * i, BL * (i + 1))
        m = dict(shared)
        m["x_sl"] = x[sl]
        m["mk_sl"] = mk[sl]
        m["mo_sl"] = mo[sl]
        in_maps.append(m)

    trace = bool(int(os.environ.get("KERNEL_TRACE", "0")))
    trace_cores = None
    if trace and int(os.environ.get("KERNEL_TRACE_ALL", "0")):
        trace_cores = list(range(N_CORES))
    res = None
    last_err = None
    for _attempt in range(3):
        try:
            res = run_bass_kernel_spmd(
                nc, in_maps, core_ids=list(range(N_CORES)), trace=trace,
                trace_cores=trace_cores,
            )
            break
        except Exception as e:  # transient NRT/device hiccups: retry
            last_err = e
    if res is None:
        raise last_err
    _cache["last_results"] = res
    out = np.concatenate(
        [res.results[i]["out"].reshape(BL, OUT) for i in range(N_CORES)], axis=0
    )
    return out.astype(np.float32)


# revision 10
# speedup vs baseline: 1.1961x; 1.1961x over previous
"""Trainium2 Bass kernel for a Bayesian (variational) single-layer LSTM.

Reference computation (B=512, S=128, H=512, IN=1, OUT=1):
    W = mu + softplus(rho) * eps            (variational reparameterization)
    u[b,s] = x[b,s] * mask_in[b,s]          (inverted input dropout)
    gates(t) = u[:,t] * W_ih + b + h(t-1) @ W_hh
    i,f,g,o = split(gates); c = f*c + i*g; h = o * tanh(c)
    out = (h(S-1) * mask_out) @ W_lin + b_lin

Strategy: data-parallel over batch (64 rows per core, 8 cores), weights
replicated.  Feature-major layout on chip: gates^T / h^T / c^T with gate
features on partitions, batch on the free dim.

    gates^T[128-feat tile m, batch] = sum_k W_hh[k-chunk, m-tile].T @ h^T[k-chunk]
                                      (+ [W_ih; b].T @ [u_s; 1])

PSUM layout (the key restructure vs the per-gate baseline): per feature
HALF j (h chunks 2j,2j+1) one full PSUM bank holds [i_j|f_j|g_j|o_j]
(each 128 cols = 2 chunks x 64 batch), so ONE sigmoid instruction per
half covers all four gates.  tanh(x)=2*sigmoid(2x)-1 scaling is folded
into the weights (g columns x2) and h is stored as h/2 (all W columns
x2, W_lin x2), so a single SIGMOID table serves the gates; the c->h
tail uses TANH (same ACT table set) with scale=2:  h/2 = 0.5*tanh(2C)*o.

PE ordering is just-in-time: per step, gx (input projection) and the
k-chunks of the EARLY h half issue first, the late half's k-chunks
last, so sigma of half 0 starts after only 16 trailing matmuls and the
sigma/c/tanh/h chain of each half overlaps the other half's matmuls.

Precision: W/h/u fp16, PSUM fp32, sigma outputs fp32, c fp32, T fp16.
"""

import os
import sys

import numpy as np

for _p in ("/opt/trn_rl_repo",):
    if _p not in sys.path:
        sys.path.insert(0, _p)

from concourse import bacc, bass, mybir, tile  # noqa: E402
from concourse.bass_utils import run_bass_kernel_spmd  # noqa: E402
from concourse.tile_rust import add_dep_helper  # noqa: E402

N_CORES = 8
B, S, H, OUT = 512, 128, 512, 1
BL = B // N_CORES            # 64 batch rows per core
G4 = 4 * H                   # 2048 gate features
KC = H // 128                # 4 contraction chunks
F32 = mybir.dt.float32
F16 = mybir.dt.float16
AF = mybir.ActivationFunctionType
OP = mybir.AluOpType

_cache = {}


def _build():
    if "built" in _cache:
        return _cache["built"]

    nc = bacc.Bacc(
        "TRN2", target_bir_lowering=False, debug=False, num_devices=N_CORES
    )

    # ---- I/O ----
    def din(name, shape):
        return nc.dram_tensor(name, shape, F32, kind="ExternalInput").ap()

    x_sl = din("x_sl", [BL, S])
    mk_sl = din("mk_sl", [BL, S])
    mo_sl = din("mo_sl", [BL, H])
    wih_mu, wih_rho, eps_ih = din("wih_mu", [1, G4]), din("wih_rho", [1, G4]), din("eps_ih", [1, G4])
    b_mu, b_rho, eps_b = din("b_mu", [1, G4]), din("b_rho", [1, G4]), din("eps_b", [1, G4])
    whh_mu, whh_rho, eps_hh = din("whh_mu", [H, G4]), din("whh_rho", [H, G4]), din("eps_hh", [H, G4])
    wlin = din("wlin", [H, OUT])
    blin = din("blin", [1, OUT])
    out_d = nc.dram_tensor("out", [BL, OUT], F32, kind="ExternalOutput").ap()
    u_scr = nc.dram_tensor("u_scr", [S, BL], F16, kind="Internal").ap()

    with tile.TileContext(nc) as tc:
        with tc.tile_pool(name="const", bufs=1) as const:
            w16 = [
                const.tile([128, G4], F16, tag=f"w16_{k}", name=f"w16_{k}")
                for k in range(KC)
            ]
            wg = const.tile([2, G4], F16, tag="wg", name="wg")
            u2 = const.tile([2, S * BL], F16, tag="u2", name="u2")
            mot = const.tile([128, KC * BL], F16, tag="mot", name="mot")
            wl16 = const.tile([128, KC], F16, tag="wl16", name="wl16")
            bl32 = const.tile([1, OUT], F32, tag="bl32", name="bl32")

            # ------------- prologue (input path first, then weights) -------
            with tc.tile_pool(name="pre", bufs=2) as pre:
                # u = x * mask_in -> transposed + flattened U2 [2, S*BL]
                xt = pre.tile([BL, S], F32, tag="xt", name="xt")
                mkt = pre.tile([BL, S], F32, tag="mkt", name="mkt")
                nc.sync.dma_start(xt[:, :], x_sl)
                nc.sync.dma_start(mkt[:, :], mk_sl)
                u16 = pre.tile([BL, S], F16, tag="u16", name="u16")
                nc.vector.tensor_mul(u16[:, :], xt[:, :], mkt[:, :])
                ut = pre.tile([S, BL], F16, tag="ut", name="ut")
                nc.sync.dma_start_transpose(ut[:, :], u16[:, :])
                nc.sync.dma_start(u_scr, ut[:, :])
                nc.sync.dma_start(
                    u2[0:1, :], u_scr.rearrange("s b -> (s b)")[None, :]
                )
                ones_row = pre.tile([1, S * BL], F16, tag="ones_row", name="ones_row")
                nc.gpsimd.memset(ones_row[:, :], 1.0)
                nc.sync.dma_start(u2[1:2, :], ones_row[:, :])

                # scale patterns: tanh(x)=2*sigmoid(2x)-1 trick needs the
                # g-gate pre-activations doubled; storing h/2 needs all
                # W_hh columns doubled (and W_lin doubled at the output).
                sc_row = pre.tile([1, G4], F32, tag="sc_row", name="sc_row", bufs=1)
                nc.gpsimd.memset(sc_row[:, :], 1.0)
                nc.gpsimd.memset(sc_row[:, 1024:1536], 2.0)
                sc_w = pre.tile([128, G4], F32, tag="sc_w", name="sc_w", bufs=1)
                nc.gpsimd.memset(sc_w[:, :], 2.0)
                nc.gpsimd.memset(sc_w[:, 1024:1536], 4.0)

                # ---- sampling, phase-batched so each ACT table set loads
                # once: all Exp ops, then all Ln ops (sets differ), then the
                # DVE chain per unit.  rho is DMA'd first so Exp starts early.
                units = [("wih", wih_rho, wih_mu, eps_ih, 1, sc_row),
                         ("bb", b_rho, b_mu, eps_b, 1, sc_row)]
                for k in range(KC):
                    rsl = slice(128 * k, 128 * (k + 1))
                    units.append((f"whh{k}", whh_rho[rsl, :], whh_mu[rsl, :],
                                  eps_hh[rsl, :], 128, sc_w))
                exs = []
                for nm, rho_ap, mu_ap, eps_ap, rows, sc in units:
                    rho = pre.tile([rows, G4], F32, tag="smp_rho",
                                   name=f"{nm}_rho")
                    nc.sync.dma_start(rho[:, :], rho_ap)
                    ex = pre.tile([rows, G4], F32, tag=f"ex_{nm}",
                                  name=f"{nm}_ex", bufs=1)
                    nc.scalar.activation(ex[:, :], rho[:, :], AF.Exp)
                    exs.append(ex)
                for ex, u in zip(exs, units):
                    nc.scalar.activation(ex[:, :], ex[:, :], AF.Ln, bias=1.0)
                outs = []
                for ex, (nm, rho_ap, mu_ap, eps_ap, rows, sc) in zip(exs, units):
                    eps = pre.tile([rows, G4], F32, tag="smp_eps",
                                   name=f"{nm}_eps")
                    mu = pre.tile([rows, G4], F32, tag="smp_mu", name=f"{nm}_mu")
                    nc.sync.dma_start(eps[:, :], eps_ap)
                    nc.sync.dma_start(mu[:, :], mu_ap)
                    nc.vector.tensor_mul(ex[:, :], ex[:, :], eps[:, :])
                    nc.vector.tensor_add(ex[:, :], ex[:, :], mu[:, :])
                    if rows == 1:
                        o = pre.tile([1, G4], F16, tag="wrow", name=f"{nm}_16")
                        nc.vector.tensor_mul(o[:, :], ex[:, :], sc[:, :])
                        outs.append(o)
                    else:
                        kk = int(nm[3:])
                        nc.vector.tensor_mul(w16[kk][:, :], ex[:, :], sc[:, :])
                nc.sync.dma_start(wg[0:1, :], outs[0][:, :])
                nc.sync.dma_start(wg[1:2, :], outs[1][:, :])

                # mask_out^T fp16, W_lin fp16, b_lin
                mo32 = pre.tile([BL, H], F32, tag="mo32", name="mo32")
                nc.sync.dma_start(mo32[:, :], mo_sl)
                mo16 = pre.tile([BL, H], F16, tag="mo16", name="mo16")
                nc.gpsimd.tensor_copy(mo16[:, :], mo32[:, :])
                for k in range(KC):
                    nc.sync.dma_start_transpose(
                        mot[:, BL * k:BL * (k + 1)], mo16[:, 128 * k:128 * (k + 1)]
                    )
                wl32 = pre.tile([128, KC], F32, tag="wl32", name="wl32")
                for k in range(KC):
                    nc.sync.dma_start(
                        wl32[:, k:k + 1], wlin[128 * k:128 * (k + 1), :]
                    )
                nc.gpsimd.tensor_scalar_mul(wl16[:, :], wl32[:, :], 2.0)
                nc.sync.dma_start(bl32[:, :], blin)

            # ------------- recurrence -------------
            # PSUM: one full bank per feature half j: [i_j|f_j|g_j|o_j],
            # each gate block 128 cols = chunks (2j,2j+1) x 64 batch.
            # m-tile m (gate gt=m//4, chunk q=m%4) -> bank q//2,
            # cols gt*128 + (q%2)*64.
            n_fill = int(os.environ.get("KERNEL_FILL", "0"))
            n_fill0 = int(os.environ.get("KERNEL_FILL0", "64"))
            with tc.tile_pool(name="work", bufs=4) as work:
              with tc.tile_pool(name="psum", bufs=2, space="PSUM") as psum:
                fill_ps = psum.tile([128, BL], F32, tag="pfill", name="pfill",
                                    bufs=1)
                h_prev = None
                c_prev = [None, None]
                for s in range(S):
                    bank = [
                        psum.tile([128, 512], F32, tag=f"pb{j}", name=f"pb{j}_{s}")
                        for j in (0, 1)
                    ]
                    u_s = u2[:, BL * s:BL * (s + 1)]

                    def out_ap(m):
                        gt, q = m // 4, m % 4
                        col = gt * 128 + (q % 2) * 64
                        return bank[q // 2][:, col:col + 64]

                    def gx_mm(m, start):
                        return nc.tensor.matmul(
                            out_ap(m),
                            wg[:, 128 * m:128 * (m + 1)],
                            u_s,
                            start=start, stop=False, skip_group_check=True,
                        )

                    def k_mm(k, m):
                        hsrc = h_prev[k // 2][:, 64 * (k % 2):64 * (k % 2 + 1)]
                        return nc.tensor.matmul(
                            out_ap(m),
                            w16[k][:, 128 * m:128 * (m + 1)],
                            hsrc,
                            start=False, stop=(k == KC - 1),
                            skip_group_check=True,
                        )

                    # gx: one opener per bank, rest accumulate.
                    openers = {}
                    for j in (0, 1):
                        m0 = 2 * j            # (gt=0, q=2j) lives in bank j
                        openers[j] = gx_mm(m0, True)
                    for m in range(16):
                        if m in (0, 2):
                            continue
                        r = gx_mm(m, False)
                        add_dep_helper(
                            r.ins, openers[(m % 4) // 2].ins,
                            reason="bank start first",
                        )
                    if h_prev is not None:
                        # JIT order: kA (chunks 0,1; early h half) for the
                        # late-half-1 m-tiles first, then half-0; kB (chunks
                        # 2,3; late h half) for half-1 first so sigma_1 --
                        # the critical chain -- unblocks after only 16
                        # trailing matmuls once h1(s-1) lands.
                        for j in (0, 1):
                            for k in (0, 1):
                                for m in range(16):
                                    if (m % 4) // 2 != j:
                                        continue
                                    r = k_mm(k, m)
                                    add_dep_helper(
                                        r.ins, openers[j].ins,
                                        reason="bank start first",
                                    )
                        for j in (0, 1):
                            for k in (2, 3):
                                for m in range(16):
                                    if (m % 4) // 2 != j:
                                        continue
                                    r = k_mm(k, m)
                                    add_dep_helper(
                                        r.ins, openers[j].ins,
                                        reason="bank start first",
                                    )
                    # step 0: gates are gx only; groups stay open (harmless
                    # with skip_group_check, matching the baseline pattern)

                    # p-state filler: the PE DVFS drops to half speed after
                    # idle gaps (>3us continuous busy needed for full clock).
                    # Cheap throwaway matmuls into a scratch bank keep the
                    # engine busy through the sigma/c/tanh chain latency so
                    # real matmuls run at ~29ns instead of ~53ns.
                    nf = n_fill0 if s < 3 else n_fill
                    for _f in range(nf):
                        nc.tensor.matmul(
                            fill_ps[:, :],
                            wg[:, 0:128],
                            u2[:, 0:BL],
                            start=True, stop=True, skip_group_check=True,
                        )

                    # one sigmoid per half over [i|f|g|o] (fp32 out);
                    # half 0 (whose h is produced first, feeding kA) first
                    sg = [None, None]
                    for j in (0, 1):
                        sg[j] = work.tile(
                            [128, 512], F32, tag=f"sg{j}", name=f"sg{j}_{s}",
                            bufs=2,
                        )
                        nc.scalar.activation(
                            sg[j][:, :], bank[j][:, :], AF.Sigmoid
                        )

                    # DVE chain per half: fc = sf*c', t = (sg-.5)*si,
                    # c = t+fc; tail: T = tanh(2C) fp16, H = (T*.5)*so fp16
                    h_new = [
                        work.tile([128, 128], F16, tag=f"hT{j}",
                                  name=f"hT{j}_{s}", bufs=2)
                        for j in (0, 1)
                    ]
                    c_new = [None, None]
                    t_t = [None, None]
                    fc_t = [None, None]
                    T_t = [None, None]

                    def chain_mul_fc(j):
                        sf = sg[j][:, 128:256]
                        if c_prev[j] is not None:
                            fc_t[j] = work.tile(
                                [128, 128], F32, tag=f"fc{j}", name=f"fc{j}_{s}",
                                bufs=2,
                            )
                            nc.vector.tensor_mul(
                                fc_t[j][:, :], sf, c_prev[j][:, :]
                            )

                    def chain_mul_t(j):
                        si = sg[j][:, 0:128]
                        sgg = sg[j][:, 256:384]
                        t_t[j] = work.tile(
                            [128, 128], F32, tag=f"t{j}", name=f"t{j}_{s}",
                            bufs=2,
                        )
                        nc.vector.scalar_tensor_tensor(
                            t_t[j][:, :], sgg, 0.5, si,
                            op0=OP.subtract, op1=OP.mult,
                        )

                    def chain_mul(j):
                        chain_mul_fc(j)
                        chain_mul_t(j)

                    def chain_add(j):
                        if c_prev[j] is None:
                            c_new[j] = t_t[j]
                            return
                        c_new[j] = work.tile(
                            [128, 128], F32, tag=f"cT{j}", name=f"cT{j}_{s}",
                            bufs=2,
                        )
                        nc.vector.tensor_add(
                            c_new[j][:, :], t_t[j][:, :], fc_t[j][:, :]
                        )

                    def chain_tanh(j):
                        T_t[j] = work.tile(
                            [128, 128], F16, tag=f"T{j}", name=f"T{j}_{s}",
                            bufs=2,
                        )
                        nc.scalar.activation(
                            T_t[j][:, :], c_new[j][:, :], AF.Tanh, scale=2.0
                        )

                    def chain_h(j):
                        so = sg[j][:, 384:512]
                        nc.vector.scalar_tensor_tensor(
                            h_new[j][:, :], T_t[j][:, :],
                            0.5, so, op0=OP.mult, op1=OP.mult,
                        )

                    # program order tuned for overlap (half 0 early, its h
                    # feeds next step's kA):
                    # DVE: fc0,t0,c0, fc1, h0, t1, c1, h1
                    # ACT: sig0, sig1, T0, T1
                    chain_mul(0)
                    chain_add(0)
                    chain_tanh(0)
                    chain_mul_fc(1)
                    chain_h(0)
                    chain_mul_t(1)
                    chain_add(1)
                    chain_tanh(1)
                    chain_h(1)
                    c_prev = c_new
                    h_prev = h_new

              # ------------- epilogue (psum pool released; reuse banks) ----
              with tc.tile_pool(name="psum2", bufs=1, space="PSUM") as psum2:
                mh = work.tile([128, KC * BL], F16, tag="mh", name="mh")
                for j in (0, 1):
                    nc.vector.tensor_mul(
                        mh[:, 128 * j:128 * (j + 1)], h_prev[j][:, :],
                        mot[:, 128 * j:128 * (j + 1)],
                    )
                pso = psum2.tile([1, BL], F32, tag="pso", name="pso", bufs=1)
                for k in range(KC):
                    nc.tensor.matmul(
                        pso[0:1, :],
                        wl16[:, k:k + 1],
                        mh[:, BL * k:BL * (k + 1)],
                        start=(k == 0), stop=(k == KC - 1),
                    )
                osb = work.tile([1, BL], F32, tag="osb", name="osb")
                nc.vector.tensor_scalar(
                    osb[:, :], pso[0:1, :], bl32[0:1, 0:1], None, op0=OP.add
                )
                nc.sync.dma_start(out_d.rearrange("b o -> o b"), osb[:, :])

    nc.compile()
    _cache["built"] = nc
    return nc


def kernel(**inputs) -> np.ndarray:
    nc = _build()
    f32 = np.float32

    def c(a):
        return np.ascontiguousarray(np.asarray(a, dtype=f32))

    shared = {
        "wih_mu": c(inputs["W_ih_mu"]).reshape(1, G4),
        "wih_rho": c(inputs["W_ih_rho"]).reshape(1, G4),
        "eps_ih": c(inputs["eps_ih"]).reshape(1, G4),
        "b_mu": c(inputs["b_mu"]).reshape(1, G4),
        "b_rho": c(inputs["b_rho"]).reshape(1, G4),
        "eps_b": c(inputs["eps_b"]).reshape(1, G4),
        "whh_mu": c(inputs["W_hh_mu"]),
        "whh_rho": c(inputs["W_hh_rho"]),
        "eps_hh": c(inputs["eps_hh"]),
        "wlin": c(inputs["W_lin"]).reshape(H, OUT),
        "blin": c(inputs["b_lin"]).reshape(1, OUT),
    }
    x = c(inputs["x"])
    mk = c(inputs["mask_in"]).reshape(B, S)
    mo = c(inputs["mask_out"])
    in_maps = []
    for i in range(N_CORES):
        sl = slice(BL * i, BL * (i + 1))
        m = dict(shared)
        m["x_sl"] = x[sl]
        m["mk_sl"] = mk[sl]
        m["mo_sl"] = mo[sl]
        in_maps.append(m)

    trace = bool(int(os.environ.get("KERNEL_TRACE", "0")))
    trace_cores = None
    if trace and int(os.environ.get("KERNEL_TRACE_ALL", "0")):
        trace_cores = list(range(N_CORES))
    res = None
    last_err = None
    for _attempt in range(3):
        try:
            res = run_bass_kernel_spmd(
                nc, in_maps, core_ids=list(range(N_CORES)), trace=trace,
                trace_cores=trace_cores,
            )
            break
        except Exception as e:  # transient NRT/device hiccups: retry
            last_err = e
    if res is None:
        raise last_err
    _cache["last_results"] = res
    out = np.concatenate(
        [res.results[i]["out"].reshape(BL, OUT) for i in range(N_CORES)], axis=0
    )
    return out.astype(np.float32)
